# revision 1
# baseline (speedup 1.0000x reference)
"""Trainium2 Bass kernel for nn_DownBlock (gnn_message_passing).

Pipeline (device does all dense linear algebra):
  1. host: replicate reference's TopK scoring (jax, bit-exact perm) + dense
     adjacency scatter-build.
  2. device launch 1 (8 cores, 4x2 grid): adj_p = A[perm] @ A[:, perm]
     -- the only part of the N^3 squaring the output actually needs.
  3. host: assemble adj_p, zero diag, degree-normalize (fold D^-1/2 A D^-1/2
     into a single pre-scaled a_normT operand).
  4. device launch 2/3 (8 cores, row-sharded, one compiled program reused):
     relu(a_norm @ (X @ w) + b) for conv1 and conv2.
  5. host: BatchNorm stats/apply + time conditioning between convs.

Matmuls run in float32r (full-rate fp32 on the PE array, ~1.7e-4 rel err).
"""

import sys
import time

sys.path.insert(0, "/opt/trn_rl_repo")

import numpy as np

N = 4096
CH = 256
KP = 2048          # nodes kept by TopKPooling
N_CORES = 8
EPS = 1e-5
RB, CB = 512, 1024  # launch-1 block: 4 row-blocks x 2 col-blocks of [2048, 2048]
RS = KP // N_CORES  # launch-2 row shard

_cache = {}


def _build_launch1():
    import concourse.tile as tile
    from concourse import bacc, mybir

    F32 = mybir.dt.float32
    F32R = mybir.dt.float32r
    nc = bacc.Bacc("TRN2", target_bir_lowering=False, debug=False, num_devices=N_CORES)
    arT = nc.dram_tensor("arT", [N, RB], F32R, kind="ExternalInput")
    ac = nc.dram_tensor("ac", [N, CB], F32R, kind="ExternalInput")
    blk = nc.dram_tensor("blk", [RB, CB], F32, kind="ExternalOutput")

    with tile.TileContext(nc) as tc:
        with (
            tc.tile_pool(name="ins", bufs=4) as pin,
            tc.tile_pool(name="outs", bufs=2) as pout,
            tc.tile_pool(name="psum", bufs=1, space="PSUM") as psum,
        ):
            accs = [psum.tile([128, CB], F32, tag=f"acc{m}", name=f"acc{m}") for m in range(4)]
            for k in range(N // 128):
                ta = pin.tile([128, RB], F32R, tag="ta", name="ta")
                nc.sync.dma_start(ta[:], arT.ap()[k * 128:(k + 1) * 128, :])
                tb = pin.tile([128, CB], F32R, tag="tb", name="tb")
                nc.sync.dma_start(tb[:], ac.ap()[k * 128:(k + 1) * 128, :])
                for m in range(4):
                    for n in range(2):
                        nc.tensor.matmul(
                            accs[m][:, n * 512:(n + 1) * 512],
                            ta[:, m * 128:(m + 1) * 128],
                            tb[:, n * 512:(n + 1) * 512],
                            start=(k == 0),
                            stop=(k == N // 128 - 1),
                        )
            for m in range(4):
                so = pout.tile([128, CB], F32, tag="so", name="so")
                nc.vector.tensor_copy(so[:], accs[m][:])
                nc.sync.dma_start(blk.ap()[m * 128:(m + 1) * 128, :], so[:])
    nc.compile()
    return nc


def _build_conv():
    import concourse.tile as tile
    from concourse import bacc, mybir

    F32 = mybir.dt.float32
    F32R = mybir.dt.float32r
    nc = bacc.Bacc("TRN2", target_bir_lowering=False, debug=False, num_devices=N_CORES)
    anT = nc.dram_tensor("anT", [KP, RS], F32R, kind="ExternalInput")
    xT = nc.dram_tensor("xT", [CH, KP], F32R, kind="ExternalInput")
    w = nc.dram_tensor("w", [CH, CH], F32R, kind="ExternalInput")
    bias = nc.dram_tensor("bias", [128, CH], F32, kind="ExternalInput")
    hr = nc.dram_tensor("hr", [RS, CH], F32, kind="ExternalOutput")

    with tile.TileContext(nc) as tc:
        with (
            tc.tile_pool(name="ins", bufs=1) as pin,
            tc.tile_pool(name="xw", bufs=1) as pxw,
            tc.tile_pool(name="outs", bufs=2) as pout,
            tc.tile_pool(name="psum", bufs=2, space="PSUM") as psum,
        ):
            xT_sb = [pin.tile([128, KP], F32R, tag=f"xT{k}", name=f"xT{k}") for k in range(2)]
            for k in range(2):
                nc.sync.dma_start(xT_sb[k][:], xT.ap()[k * 128:(k + 1) * 128, :])
            w_sb = [pin.tile([128, CH], F32R, tag=f"w{k}", name=f"w{k}") for k in range(2)]
            for k in range(2):
                nc.sync.dma_start(w_sb[k][:], w.ap()[k * 128:(k + 1) * 128, :])
            anT_sb = [pin.tile([128, RS], F32R, tag=f"anT{k}", name=f"anT{k}") for k in range(16)]
            for k in range(16):
                nc.sync.dma_start(anT_sb[k][:], anT.ap()[k * 128:(k + 1) * 128, :])
            bias_sb = pin.tile([128, CH], F32, tag="bias", name="bias")
            nc.sync.dma_start(bias_sb[:], bias.ap())

            # stage A: XW = X @ w, laid out as 16 tiles [128, CH] (partition = node)
            xw_sb = []
            for m in range(16):
                ps = psum.tile([128, CH], F32, tag="psA", name="psA")
                for k in range(2):
                    nc.tensor.matmul(
                        ps[:],
                        xT_sb[k][:, m * 128:(m + 1) * 128],
                        w_sb[k][:],
                        start=(k == 0),
                        stop=(k == 1),
                    )
                sb = pxw.tile([128, CH], F32R, tag=f"xw{m}", name=f"xw{m}")
                nc.vector.tensor_copy(sb[:], ps[:])
                xw_sb.append(sb)

            # stage B: H[R_c] = a_norm[R_c, :] @ XW + b, relu
            for m2 in range(RS // 128):
                ps = psum.tile([128, CH], F32, tag="psB", name="psB")
                for k in range(16):
                    nc.tensor.matmul(
                        ps[:],
                        anT_sb[k][:, m2 * 128:(m2 + 1) * 128],
                        xw_sb[k][:],
                        start=(k == 0),
                        stop=(k == 15),
                    )
                ob = pout.tile([128, CH], F32, tag="ob", name="ob")
                nc.vector.tensor_add(ob[:], ps[:], bias_sb[:])
                nc.vector.tensor_relu(ob[:], ob[:])
                nc.sync.dma_start(hr.ap()[m2 * 128:(m2 + 1) * 128, :], ob[:])
    nc.compile()
    return nc


def _programs():
    if "l1" not in _cache:
        _cache["l1"] = _build_launch1()
        _cache["conv"] = _build_conv()
    return _cache["l1"], _cache["conv"]


def _run_spmd(nc, in_maps, tries=3):
    from concourse.bass_utils import run_bass_kernel_spmd

    last = None
    for attempt in range(tries):
        try:
            return run_bass_kernel_spmd(nc, in_maps, list(range(N_CORES)))
        except Exception as e:  # transient NRT/axon failures: retry
            last = e
            time.sleep(2.0)
    raise last


def _topk_and_adj(x, edge_index, edge_weight, pool_w):
    """Replicate the reference's scoring/top_k with the same jax ops so the
    integer perm output matches bit-exactly; build the dense adjacency."""
    import jax
    import jax.numpy as jnp

    score = jnp.tanh(jnp.asarray(x) @ jnp.asarray(pool_w) / jnp.linalg.norm(jnp.asarray(pool_w)))
    try:
        top_score_j, perm_j = jax.lax.top_k(score, KP)
        top_score = np.asarray(top_score_j)
        perm = np.asarray(perm_j)
    except Exception:
        s = np.asarray(score)
        perm = np.argsort(-s, kind="stable")[:KP].astype(np.int32)
        top_score = s[perm]

    A = np.zeros((N, N), np.float32)
    np.add.at(A, (np.asarray(edge_index[0]), np.asarray(edge_index[1])), np.asarray(edge_weight, np.float32))
    np.fill_diagonal(A, 1.0)
    return perm, top_score, A


def _conv_in_maps(a_normT, xT, w, b):
    bias_b = np.broadcast_to(np.asarray(b, np.float32), (128, CH)).copy()
    w = np.ascontiguousarray(np.asarray(w, np.float32))
    maps = []
    for c in range(N_CORES):
        maps.append({
            "anT": np.ascontiguousarray(a_normT[:, c * RS:(c + 1) * RS]),
            "xT": xT,
            "w": w,
            "bias": bias_b,
        })
    return maps


def kernel(x, edge_index, edge_weight, batch, t,
           conv1_w, conv1_b, conv2_w, conv2_b,
           bn1_gamma, bn1_beta, bn2_gamma, bn2_beta,
           pool_w, time_w, time_b):
    x = np.asarray(x, np.float32)
    batch = np.asarray(batch)

    perm, top_score, A = _topk_and_adj(x, edge_index, edge_weight, pool_w)

    nc_l1, nc_conv = _programs()

    # ---- launch 1: adj_p = A[perm] @ A[:, perm], 4x2 blocks ----
    Ap = A[perm]                      # [2048, 4096]
    Ac = np.ascontiguousarray(A[:, perm])  # [4096, 2048]
    in_maps = []
    for c in range(N_CORES):
        p, q = divmod(c, 2)
        in_maps.append({
            "arT": np.ascontiguousarray(Ap[p * RB:(p + 1) * RB, :].T),
            "ac": np.ascontiguousarray(Ac[:, q * CB:(q + 1) * CB]),
        })
    res = _run_spmd(nc_l1, in_maps)
    adj_p = np.empty((KP, KP), np.float32)
    for c in range(N_CORES):
        p, q = divmod(c, 2)
        adj_p[p * RB:(p + 1) * RB, q * CB:(q + 1) * CB] = res.results[c]["blk"]
    np.fill_diagonal(adj_p, 0.0)

    # ---- host: GCN normalization, folded into one operand ----
    deg = adj_p.sum(axis=1, dtype=np.float32) + np.float32(2.0)
    dinv = np.where(deg > 0, deg.astype(np.float32) ** -0.5, 0.0).astype(np.float32)
    a_normT = adj_p.T * dinv[None, :]          # dinv_i on rows of a_norm
    a_normT *= dinv[:, None]                   # dinv_j on cols of a_norm
    idx = np.arange(KP)
    a_normT[idx, idx] = 2.0 * dinv * dinv      # diag of a is 2.0
    a_normT = np.ascontiguousarray(a_normT, np.float32)

    # ---- launch 2: conv1 ----
    xp = x[perm] * top_score[:, None].astype(np.float32)
    xT = np.ascontiguousarray(xp.T.astype(np.float32))
    res = _run_spmd(nc_conv, _conv_in_maps(a_normT, xT, conv1_w, conv1_b))
    h1 = np.concatenate([res.results[c]["hr"] for c in range(N_CORES)], axis=0)

    # host BN1 + time conditioning
    m1 = h1.mean(axis=0, dtype=np.float32)
    v1 = h1.var(axis=0, dtype=np.float32)
    h = (h1 - m1) * (1.0 / np.sqrt(v1 + np.float32(EPS))) * np.asarray(bn1_gamma, np.float32) + np.asarray(bn1_beta, np.float32)
    tvec = np.maximum(np.asarray(t, np.float32) @ np.asarray(time_w, np.float32) + np.asarray(time_b, np.float32), 0.0)
    h = h + tvec

    # ---- launch 3: conv2 (same program) ----
    hT = np.ascontiguousarray(h.T.astype(np.float32))
    res = _run_spmd(nc_conv, _conv_in_maps(a_normT, hT, conv2_w, conv2_b))
    h2 = np.concatenate([res.results[c]["hr"] for c in range(N_CORES)], axis=0)

    m2 = h2.mean(axis=0, dtype=np.float32)
    v2 = h2.var(axis=0, dtype=np.float32)
    h_out = (h2 - m2) * (1.0 / np.sqrt(v2 + np.float32(EPS))) * np.asarray(bn2_gamma, np.float32) + np.asarray(bn2_beta, np.float32)

    batch_p = batch[perm]
    return h_out.astype(np.float32), adj_p, batch_p, perm


# revision 4
# speedup vs baseline: 1.0958x; 1.0958x over previous
"""Trainium2 Bass kernel for nn_DownBlock (gnn_message_passing).

Pipeline (device does all dense linear algebra):
  1. host: replicate reference's TopK scoring (jax, bit-exact perm) + dense
     adjacency scatter-build.
  2. device launch 1 (8 cores, 4x2 grid): adj_p = A[perm] @ A[:, perm]
     -- the only part of the N^3 squaring the output actually needs.
  3. host: assemble adj_p, zero diag, degree-normalize (fold D^-1/2 A D^-1/2
     into a single pre-scaled a_normT operand).
  4. device launch 2/3 (8 cores, row-sharded, one compiled program reused):
     relu(a_norm @ (X @ w) + b) for conv1 and conv2.
  5. host: BatchNorm stats/apply + time conditioning between convs.

Matmuls run in float32r (full-rate fp32 on the PE array, ~1.7e-4 rel err).
"""

import sys
import time


import numpy as np

N = 4096
CH = 256
KP = 2048          # nodes kept by TopKPooling
N_CORES = 8
EPS = 1e-5
RB, CB = 512, 1024  # launch-1 block: 4 row-blocks x 2 col-blocks of [2048, 2048]
RS = KP // N_CORES  # launch-2 row shard

_cache = {}


def _build_launch1():
    import concourse.tile as tile
    from concourse import bacc, mybir

    F32 = mybir.dt.float32
    F32R = mybir.dt.float32r
    BF16 = mybir.dt.bfloat16
    nc = bacc.Bacc("TRN2", target_bir_lowering=False, debug=False, num_devices=N_CORES)
    arT = nc.dram_tensor("arT", [N, RB], F32R, kind="ExternalInput")
    ac = nc.dram_tensor("ac", [N, CB], BF16, kind="ExternalInput")
    blk = nc.dram_tensor("blk", [RB, CB], F32, kind="ExternalOutput")

    with tile.TileContext(nc) as tc:
        with (
            tc.tile_pool(name="ins", bufs=8) as pin,
            tc.tile_pool(name="outs", bufs=2) as pout,
            tc.tile_pool(name="psum", bufs=1, space="PSUM") as psum,
        ):
            accs = [psum.tile([128, CB], F32, tag=f"acc{m}", name=f"acc{m}") for m in range(4)]
            for k in range(N // 128):
                ta = pin.tile([128, RB], F32R, tag="ta", name="ta")
                nc.sync.dma_start(ta[:], arT.ap()[k * 128:(k + 1) * 128, :])
                # ac ships as bf16 (halves the dominant DMA stream) and is
                # widened to f32r on DVE; arT stays f32r for precision.
                tbh = pin.tile([128, CB], BF16, tag="tbh", name="tbh")
                nc.sync.dma_start(tbh[:], ac.ap()[k * 128:(k + 1) * 128, :])
                tb = pin.tile([128, CB], F32R, tag="tb", name="tb")
                nc.vector.tensor_copy(tb[:], tbh[:])
                for m in range(4):
                    for n in range(2):
                        nc.tensor.matmul(
                            accs[m][:, n * 512:(n + 1) * 512],
                            ta[:, m * 128:(m + 1) * 128],
                            tb[:, n * 512:(n + 1) * 512],
                            start=(k == 0),
                            stop=(k == N // 128 - 1),
                        )
            for m in range(4):
                so = pout.tile([128, CB], F32, tag="so", name="so")
                nc.vector.tensor_copy(so[:], accs[m][:])
                nc.sync.dma_start(blk.ap()[m * 128:(m + 1) * 128, :], so[:])
    nc.compile()
    return nc


def _build_conv():
    import concourse.tile as tile
    from concourse import bacc, mybir

    F32 = mybir.dt.float32
    F32R = mybir.dt.float32r
    nc = bacc.Bacc("TRN2", target_bir_lowering=False, debug=False, num_devices=N_CORES)
    anT = nc.dram_tensor("anT", [KP, RS], F32R, kind="ExternalInput")
    xT = nc.dram_tensor("xT", [CH, KP], F32R, kind="ExternalInput")
    w = nc.dram_tensor("w", [CH, CH], F32R, kind="ExternalInput")
    bias = nc.dram_tensor("bias", [128, CH], F32, kind="ExternalInput")
    hr = nc.dram_tensor("hr", [RS, CH], F32, kind="ExternalOutput")

    with tile.TileContext(nc) as tc:
        with (
            tc.tile_pool(name="ins", bufs=1) as pin,
            tc.tile_pool(name="xw", bufs=1) as pxw,
            tc.tile_pool(name="outs", bufs=2) as pout,
            tc.tile_pool(name="psum", bufs=2, space="PSUM") as psum,
        ):
            xT_sb = [pin.tile([128, KP], F32R, tag=f"xT{k}", name=f"xT{k}") for k in range(2)]
            for k in range(2):
                nc.sync.dma_start(xT_sb[k][:], xT.ap()[k * 128:(k + 1) * 128, :])
            w_sb = [pin.tile([128, CH], F32R, tag=f"w{k}", name=f"w{k}") for k in range(2)]
            for k in range(2):
                nc.sync.dma_start(w_sb[k][:], w.ap()[k * 128:(k + 1) * 128, :])
            anT_sb = [pin.tile([128, RS], F32R, tag=f"anT{k}", name=f"anT{k}") for k in range(16)]
            for k in range(16):
                nc.sync.dma_start(anT_sb[k][:], anT.ap()[k * 128:(k + 1) * 128, :])
            bias_sb = pin.tile([128, CH], F32, tag="bias", name="bias")
            nc.sync.dma_start(bias_sb[:], bias.ap())

            # stage A: XW = X @ w, laid out as 16 tiles [128, CH] (partition = node)
            xw_sb = []
            for m in range(16):
                ps = psum.tile([128, CH], F32, tag="psA", name="psA")
                for k in range(2):
                    nc.tensor.matmul(
                        ps[:],
                        xT_sb[k][:, m * 128:(m + 1) * 128],
                        w_sb[k][:],
                        start=(k == 0),
                        stop=(k == 1),
                    )
                sb = pxw.tile([128, CH], F32R, tag=f"xw{m}", name=f"xw{m}")
                nc.vector.tensor_copy(sb[:], ps[:])
                xw_sb.append(sb)

            # stage B: H[R_c] = a_norm[R_c, :] @ XW + b, relu
            for m2 in range(RS // 128):
                ps = psum.tile([128, CH], F32, tag="psB", name="psB")
                for k in range(16):
                    nc.tensor.matmul(
                        ps[:],
                        anT_sb[k][:, m2 * 128:(m2 + 1) * 128],
                        xw_sb[k][:],
                        start=(k == 0),
                        stop=(k == 15),
                    )
                ob = pout.tile([128, CH], F32, tag="ob", name="ob")
                nc.vector.tensor_add(ob[:], ps[:], bias_sb[:])
                nc.vector.tensor_relu(ob[:], ob[:])
                nc.sync.dma_start(hr.ap()[m2 * 128:(m2 + 1) * 128, :], ob[:])
    nc.compile()
    return nc


def _programs():
    if "l1" not in _cache:
        _cache["l1"] = _build_launch1()
        _cache["conv"] = _build_conv()
    return _cache["l1"], _cache["conv"]


def _run_spmd(nc, in_maps, tries=3):
    from concourse.bass_utils import run_bass_kernel_spmd

    last = None
    for attempt in range(tries):
        try:
            return run_bass_kernel_spmd(nc, in_maps, list(range(N_CORES)))
        except Exception as e:  # transient NRT/axon failures: retry
            last = e
            time.sleep(2.0)
    raise last


def _topk_and_adj(x, edge_index, edge_weight, pool_w):
    """Replicate the reference's scoring/top_k with the same jax ops so the
    integer perm output matches bit-exactly; build the dense adjacency."""
    import jax
    import jax.numpy as jnp

    score = jnp.tanh(jnp.asarray(x) @ jnp.asarray(pool_w) / jnp.linalg.norm(jnp.asarray(pool_w)))
    try:
        top_score_j, perm_j = jax.lax.top_k(score, KP)
        top_score = np.asarray(top_score_j)
        perm = np.asarray(perm_j)
    except Exception:
        s = np.asarray(score)
        perm = np.argsort(-s, kind="stable")[:KP].astype(np.int32)
        top_score = s[perm]

    A = np.zeros((N, N), np.float32)
    np.add.at(A, (np.asarray(edge_index[0]), np.asarray(edge_index[1])), np.asarray(edge_weight, np.float32))
    np.fill_diagonal(A, 1.0)
    return perm, top_score, A


def _conv_in_maps(a_normT, xT, w, b):
    bias_b = np.broadcast_to(np.asarray(b, np.float32), (128, CH)).copy()
    w = np.ascontiguousarray(np.asarray(w, np.float32))
    maps = []
    for c in range(N_CORES):
        maps.append({
            "anT": np.ascontiguousarray(a_normT[:, c * RS:(c + 1) * RS]),
            "xT": xT,
            "w": w,
            "bias": bias_b,
        })
    return maps


def kernel(x, edge_index, edge_weight, batch, t,
           conv1_w, conv1_b, conv2_w, conv2_b,
           bn1_gamma, bn1_beta, bn2_gamma, bn2_beta,
           pool_w, time_w, time_b):
    x = np.asarray(x, np.float32)
    batch = np.asarray(batch)

    perm, top_score, A = _topk_and_adj(x, edge_index, edge_weight, pool_w)

    nc_l1, nc_conv = _programs()

    # ---- launch 1: adj_p = A[perm] @ A[:, perm], 4x2 blocks ----
    import ml_dtypes

    Ap = A[perm]                      # [2048, 4096]
    Ac = np.ascontiguousarray(A[:, perm])  # [4096, 2048]
    ac_blocks = [np.ascontiguousarray(Ac[:, q * CB:(q + 1) * CB]).astype(ml_dtypes.bfloat16) for q in range(2)]
    in_maps = []
    for c in range(N_CORES):
        p, q = divmod(c, 2)
        in_maps.append({
            "arT": np.ascontiguousarray(Ap[p * RB:(p + 1) * RB, :].T),
            "ac": ac_blocks[q],
        })
    res = _run_spmd(nc_l1, in_maps)
    adj_p = np.empty((KP, KP), np.float32)
    for c in range(N_CORES):
        p, q = divmod(c, 2)
        adj_p[p * RB:(p + 1) * RB, q * CB:(q + 1) * CB] = res.results[c]["blk"]
    np.fill_diagonal(adj_p, 0.0)

    # ---- host: GCN normalization, folded into one operand ----
    deg = adj_p.sum(axis=1, dtype=np.float32) + np.float32(2.0)
    dinv = np.where(deg > 0, deg.astype(np.float32) ** -0.5, 0.0).astype(np.float32)
    a_normT = adj_p.T * dinv[None, :]          # dinv_i on rows of a_norm
    a_normT *= dinv[:, None]                   # dinv_j on cols of a_norm
    idx = np.arange(KP)
    a_normT[idx, idx] = 2.0 * dinv * dinv      # diag of a is 2.0
    a_normT = np.ascontiguousarray(a_normT, np.float32)

    # ---- launch 2: conv1 ----
    xp = x[perm] * top_score[:, None].astype(np.float32)
    xT = np.ascontiguousarray(xp.T.astype(np.float32))
    res = _run_spmd(nc_conv, _conv_in_maps(a_normT, xT, conv1_w, conv1_b))
    h1 = np.concatenate([res.results[c]["hr"] for c in range(N_CORES)], axis=0)

    # host BN1 + time conditioning
    m1 = h1.mean(axis=0, dtype=np.float32)
    v1 = h1.var(axis=0, dtype=np.float32)
    h = (h1 - m1) * (1.0 / np.sqrt(v1 + np.float32(EPS))) * np.asarray(bn1_gamma, np.float32) + np.asarray(bn1_beta, np.float32)
    tvec = np.maximum(np.asarray(t, np.float32) @ np.asarray(time_w, np.float32) + np.asarray(time_b, np.float32), 0.0)
    h = h + tvec

    # ---- launch 3: conv2 (same program) ----
    hT = np.ascontiguousarray(h.T.astype(np.float32))
    res = _run_spmd(nc_conv, _conv_in_maps(a_normT, hT, conv2_w, conv2_b))
    h2 = np.concatenate([res.results[c]["hr"] for c in range(N_CORES)], axis=0)

    m2 = h2.mean(axis=0, dtype=np.float32)
    v2 = h2.var(axis=0, dtype=np.float32)
    h_out = (h2 - m2) * (1.0 / np.sqrt(v2 + np.float32(EPS))) * np.asarray(bn2_gamma, np.float32) + np.asarray(bn2_beta, np.float32)

    batch_p = batch[perm]
    return h_out.astype(np.float32), adj_p, batch_p, perm


# revision 7
# speedup vs baseline: 1.1579x; 1.0567x over previous
"""Trainium2 Bass kernel for nn_DownBlock (gnn_message_passing).

Pipeline (device does all dense linear algebra):
  1. host: replicate reference's TopK scoring (jax, bit-exact perm) + dense
     adjacency scatter-build.
  2. device launch 1 (8 cores, 4x2 grid): adj_p = A[perm] @ A[:, perm]
     -- the only part of the N^3 squaring the output actually needs.
  3. host: assemble adj_p, zero diag, degree-normalize (fold D^-1/2 A D^-1/2
     into a single pre-scaled a_normT operand).
  4. device launch 2/3 (8 cores, row-sharded, one compiled program reused):
     relu(a_norm @ (X @ w) + b) for conv1 and conv2.
  5. host: BatchNorm stats/apply + time conditioning between convs.

Matmuls run in float32r (full-rate fp32 on the PE array, ~1.7e-4 rel err).
"""

import sys
import time


import numpy as np

N = 4096
CH = 256
KP = 2048          # nodes kept by TopKPooling
N_CORES = 8
EPS = 1e-5
RB, CB = 512, 1024  # launch-1 block: 4 row-blocks x 2 col-blocks of [2048, 2048]
RS = KP // N_CORES  # launch-2 row shard

_cache = {}


def _build_launch1():
    import concourse.tile as tile
    from concourse import bacc, mybir

    F32 = mybir.dt.float32
    F16 = mybir.dt.float16
    nc = bacc.Bacc("TRN2", target_bir_lowering=False, debug=False, num_devices=N_CORES)
    # Both operands ship and multiply as fp16: the adjacency values are
    # range-safe (0..~4), the PE is exact on fp16 operands with fp32 PSUM
    # accumulation, and fp16's 10 mantissa bits beat f32r's internal rounding
    # while halving the dominant DMA stream.
    arT = nc.dram_tensor("arT", [N, RB], F16, kind="ExternalInput")
    ac = nc.dram_tensor("ac", [N, CB], F16, kind="ExternalInput")
    blk = nc.dram_tensor("blk", [RB, CB], F32, kind="ExternalOutput")

    with tile.TileContext(nc) as tc:
        with (
            tc.tile_pool(name="ins", bufs=8) as pin,
            tc.tile_pool(name="outs", bufs=2) as pout,
            tc.tile_pool(name="psum", bufs=1, space="PSUM") as psum,
        ):
            accs = [psum.tile([128, CB], F32, tag=f"acc{m}", name=f"acc{m}") for m in range(4)]
            for k in range(N // 128):
                ta = pin.tile([128, RB], F16, tag="ta", name="ta")
                nc.sync.dma_start(ta[:], arT.ap()[k * 128:(k + 1) * 128, :])
                tb = pin.tile([128, CB], F16, tag="tb", name="tb")
                nc.sync.dma_start(tb[:], ac.ap()[k * 128:(k + 1) * 128, :])
                for m in range(4):
                    for n in range(2):
                        nc.tensor.matmul(
                            accs[m][:, n * 512:(n + 1) * 512],
                            ta[:, m * 128:(m + 1) * 128],
                            tb[:, n * 512:(n + 1) * 512],
                            start=(k == 0),
                            stop=(k == N // 128 - 1),
                        )
            for m in range(4):
                so = pout.tile([128, CB], F32, tag="so", name="so")
                nc.vector.tensor_copy(so[:], accs[m][:])
                nc.sync.dma_start(blk.ap()[m * 128:(m + 1) * 128, :], so[:])
    nc.compile()
    return nc


def _build_conv():
    import concourse.tile as tile
    from concourse import bacc, mybir

    F32 = mybir.dt.float32
    F32R = mybir.dt.float32r
    nc = bacc.Bacc("TRN2", target_bir_lowering=False, debug=False, num_devices=N_CORES)
    anT = nc.dram_tensor("anT", [KP, RS], F32R, kind="ExternalInput")
    xT = nc.dram_tensor("xT", [CH, KP], F32R, kind="ExternalInput")
    w = nc.dram_tensor("w", [CH, CH], F32R, kind="ExternalInput")
    bias = nc.dram_tensor("bias", [128, CH], F32, kind="ExternalInput")
    hr = nc.dram_tensor("hr", [RS, CH], F32, kind="ExternalOutput")

    with tile.TileContext(nc) as tc:
        with (
            tc.tile_pool(name="ins", bufs=1) as pin,
            tc.tile_pool(name="xw", bufs=1) as pxw,
            tc.tile_pool(name="outs", bufs=2) as pout,
            tc.tile_pool(name="psum", bufs=2, space="PSUM") as psum,
        ):
            xT_sb = [pin.tile([128, KP], F32R, tag=f"xT{k}", name=f"xT{k}") for k in range(2)]
            for k in range(2):
                nc.sync.dma_start(xT_sb[k][:], xT.ap()[k * 128:(k + 1) * 128, :])
            w_sb = [pin.tile([128, CH], F32R, tag=f"w{k}", name=f"w{k}") for k in range(2)]
            for k in range(2):
                nc.sync.dma_start(w_sb[k][:], w.ap()[k * 128:(k + 1) * 128, :])
            anT_sb = [pin.tile([128, RS], F32R, tag=f"anT{k}", name=f"anT{k}") for k in range(16)]
            for k in range(16):
                nc.sync.dma_start(anT_sb[k][:], anT.ap()[k * 128:(k + 1) * 128, :])
            bias_sb = pin.tile([128, CH], F32, tag="bias", name="bias")
            nc.sync.dma_start(bias_sb[:], bias.ap())

            # stage A: XW = X @ w, laid out as 16 tiles [128, CH] (partition = node)
            xw_sb = []
            for m in range(16):
                ps = psum.tile([128, CH], F32, tag="psA", name="psA")
                for k in range(2):
                    nc.tensor.matmul(
                        ps[:],
                        xT_sb[k][:, m * 128:(m + 1) * 128],
                        w_sb[k][:],
                        start=(k == 0),
                        stop=(k == 1),
                    )
                sb = pxw.tile([128, CH], F32R, tag=f"xw{m}", name=f"xw{m}")
                nc.vector.tensor_copy(sb[:], ps[:])
                xw_sb.append(sb)

            # stage B: H[R_c] = a_norm[R_c, :] @ XW + b, relu
            for m2 in range(RS // 128):
                ps = psum.tile([128, CH], F32, tag="psB", name="psB")
                for k in range(16):
                    nc.tensor.matmul(
                        ps[:],
                        anT_sb[k][:, m2 * 128:(m2 + 1) * 128],
                        xw_sb[k][:],
                        start=(k == 0),
                        stop=(k == 15),
                    )
                ob = pout.tile([128, CH], F32, tag="ob", name="ob")
                nc.vector.tensor_add(ob[:], ps[:], bias_sb[:])
                nc.vector.tensor_relu(ob[:], ob[:])
                nc.sync.dma_start(hr.ap()[m2 * 128:(m2 + 1) * 128, :], ob[:])
    nc.compile()
    return nc


def _programs():
    if "l1" not in _cache:
        _cache["l1"] = _build_launch1()
        _cache["conv"] = _build_conv()
    return _cache["l1"], _cache["conv"]


def _run_spmd(nc, in_maps, tries=3):
    from concourse.bass_utils import run_bass_kernel_spmd

    last = None
    for attempt in range(tries):
        try:
            return run_bass_kernel_spmd(nc, in_maps, list(range(N_CORES)))
        except Exception as e:  # transient NRT/axon failures: retry
            last = e
            time.sleep(2.0)
    raise last


def _topk_and_adj(x, edge_index, edge_weight, pool_w):
    """Replicate the reference's scoring/top_k with the same jax ops so the
    integer perm output matches bit-exactly; build the dense adjacency."""
    import jax
    import jax.numpy as jnp

    score = jnp.tanh(jnp.asarray(x) @ jnp.asarray(pool_w) / jnp.linalg.norm(jnp.asarray(pool_w)))
    try:
        top_score_j, perm_j = jax.lax.top_k(score, KP)
        top_score = np.asarray(top_score_j)
        perm = np.asarray(perm_j)
    except Exception:
        s = np.asarray(score)
        perm = np.argsort(-s, kind="stable")[:KP].astype(np.int32)
        top_score = s[perm]

    A = np.zeros((N, N), np.float32)
    np.add.at(A, (np.asarray(edge_index[0]), np.asarray(edge_index[1])), np.asarray(edge_weight, np.float32))
    np.fill_diagonal(A, 1.0)
    return perm, top_score, A


def _conv_in_maps(a_normT, xT, w, b):
    bias_b = np.broadcast_to(np.asarray(b, np.float32), (128, CH)).copy()
    w = np.ascontiguousarray(np.asarray(w, np.float32))
    maps = []
    for c in range(N_CORES):
        maps.append({
            "anT": np.ascontiguousarray(a_normT[:, c * RS:(c + 1) * RS]),
            "xT": xT,
            "w": w,
            "bias": bias_b,
        })
    return maps


def kernel(x, edge_index, edge_weight, batch, t,
           conv1_w, conv1_b, conv2_w, conv2_b,
           bn1_gamma, bn1_beta, bn2_gamma, bn2_beta,
           pool_w, time_w, time_b):
    x = np.asarray(x, np.float32)
    batch = np.asarray(batch)

    perm, top_score, A = _topk_and_adj(x, edge_index, edge_weight, pool_w)

    nc_l1, nc_conv = _programs()

    # ---- launch 1: adj_p = A[perm] @ A[:, perm], 4x2 blocks ----
    Ap = A[perm]                      # [2048, 4096]
    Ac = np.ascontiguousarray(A[:, perm])  # [4096, 2048]
    ac_blocks = [np.ascontiguousarray(Ac[:, q * CB:(q + 1) * CB]).astype(np.float16) for q in range(2)]
    arT_blocks = [np.ascontiguousarray(Ap[p * RB:(p + 1) * RB, :].T).astype(np.float16) for p in range(4)]
    in_maps = []
    for c in range(N_CORES):
        p, q = divmod(c, 2)
        in_maps.append({
            "arT": arT_blocks[p],
            "ac": ac_blocks[q],
        })
    res = _run_spmd(nc_l1, in_maps)
    adj_p = np.empty((KP, KP), np.float32)
    for c in range(N_CORES):
        p, q = divmod(c, 2)
        adj_p[p * RB:(p + 1) * RB, q * CB:(q + 1) * CB] = res.results[c]["blk"]
    np.fill_diagonal(adj_p, 0.0)

    # ---- host: GCN normalization, folded into one operand ----
    deg = adj_p.sum(axis=1, dtype=np.float32) + np.float32(2.0)
    dinv = np.where(deg > 0, deg.astype(np.float32) ** -0.5, 0.0).astype(np.float32)
    a_normT = adj_p.T * dinv[None, :]          # dinv_i on rows of a_norm
    a_normT *= dinv[:, None]                   # dinv_j on cols of a_norm
    idx = np.arange(KP)
    a_normT[idx, idx] = 2.0 * dinv * dinv      # diag of a is 2.0
    a_normT = np.ascontiguousarray(a_normT, np.float32)

    # ---- launch 2: conv1 ----
    xp = x[perm] * top_score[:, None].astype(np.float32)
    xT = np.ascontiguousarray(xp.T.astype(np.float32))
    res = _run_spmd(nc_conv, _conv_in_maps(a_normT, xT, conv1_w, conv1_b))
    h1 = np.concatenate([res.results[c]["hr"] for c in range(N_CORES)], axis=0)

    # host BN1 + time conditioning
    m1 = h1.mean(axis=0, dtype=np.float32)
    v1 = h1.var(axis=0, dtype=np.float32)
    h = (h1 - m1) * (1.0 / np.sqrt(v1 + np.float32(EPS))) * np.asarray(bn1_gamma, np.float32) + np.asarray(bn1_beta, np.float32)
    tvec = np.maximum(np.asarray(t, np.float32) @ np.asarray(time_w, np.float32) + np.asarray(time_b, np.float32), 0.0)
    h = h + tvec

    # ---- launch 3: conv2 (same program) ----
    hT = np.ascontiguousarray(h.T.astype(np.float32))
    res = _run_spmd(nc_conv, _conv_in_maps(a_normT, hT, conv2_w, conv2_b))
    h2 = np.concatenate([res.results[c]["hr"] for c in range(N_CORES)], axis=0)

    m2 = h2.mean(axis=0, dtype=np.float32)
    v2 = h2.var(axis=0, dtype=np.float32)
    h_out = (h2 - m2) * (1.0 / np.sqrt(v2 + np.float32(EPS))) * np.asarray(bn2_gamma, np.float32) + np.asarray(bn2_beta, np.float32)

    batch_p = batch[perm]
    return h_out.astype(np.float32), adj_p, batch_p, perm
